# revision 25
# baseline (speedup 1.0000x reference)
"""Trainium2 Bass kernel for AttLayer pooling (B=32, T=2048, D=1024, H=5).

Math (equivalent to reference up to exact cancellation of the softmax
normalization): since |tanh| <= 1, scores s[b,t] are bounded by ||uw||_1, so
exp needs no max-subtraction, and the masked renormalization cancels the
softmax denominator:

    out[b,:] = sum_t x[b,t,:] * g[b,t] / sum_t g[b,t]
    g[b,t]   = exp(s[b,t] - 256) with s' = s + 256*mask  (mask fold: masked
               entries get exp(s-256) ~ 1e-110 ~ 0; unmasked are exact)
    s[b,t]   = sum_h tanh( (x @ W)[b,t,h] + fea[b,t]*Wf[h] + bw[h] ) * uw[h]

v2 design (engine budget per core, 4 batches):
  - x arrives as bf16 via GpSimd casting DMA (f32 HBM -> bf16 SBUF), so no
    cast ops on compute engines.  DMA ~94us (streaming floor) is the target
    critical path.
  - PE (~86us): transposes x (bf16, 77ns/128x128), scores GEMM W.T @ xT
    ([5,512] per chunk), fea K=1 matmul, uw matmul with M=1 (lhsT=[6,1])
    giving s' as a psum ROW [1,512], final [128,8] output transpose.
  - ACT (~58us): half the psum->sbuf xT copies, tanh, exp (bias=-256) with
    accum_out giving the per-group denominator for free.
  - DVE (~64us): other half of copies + the whole num reduction:
    scalar_tensor_tensor(xt * g_bcast, accum_out) per (chunk, group) -> num
    columns; num never touches the PE.
  - GpSimd (~30us): casting DMAs, mask u8->f32 DMA straight into tanh row 5,
    per-group partition_broadcast of g.
"""

import sys

sys.path.insert(0, "/opt/trn_rl_repo")

import numpy as np

import concourse.bass as bass
import concourse.mybir as mybir
import concourse.tile as tile
from concourse import bacc
from concourse.masks import make_identity

F32 = mybir.dt.float32
F32R = mybir.dt.float32r
BF16 = mybir.dt.bfloat16
U8 = mybir.dt.uint8
AF = mybir.ActivationFunctionType

P = 128          # partitions / t-tile size
D = 1024         # feature dim
H = 5            # attention hidden dim
NCHUNK = D // P  # 8 d-chunks
MC = 256.0       # mask fold constant: s' = s + MC*mask, exp bias -MC


def build_kernel(b_shard: int, T: int, t_grp: int = 512):
    assert T % t_grp == 0 and t_grp % P == 0
    jg = t_grp // P              # tiles per group (4)
    n_grp = T // t_grp           # groups per batch (4)

    nc = bacc.Bacc(None)

    x_temp = nc.dram_tensor("x_temp", [b_shard, T, D], F32, kind="ExternalInput")
    x_fea = nc.dram_tensor("x_fea", [b_shard, T], F32R, kind="ExternalInput")
    mask = nc.dram_tensor("mask", [b_shard, T], U8, kind="ExternalInput")
    W_temp = nc.dram_tensor("W_temp", [D, H], F32, kind="ExternalInput")
    W_fea = nc.dram_tensor("W_fea", [1, H], F32R, kind="ExternalInput")
    bw = nc.dram_tensor("bw", [H], F32, kind="ExternalInput")
    uw = nc.dram_tensor("uw", [H], F32, kind="ExternalInput")
    out = nc.dram_tensor("out", [b_shard, D], F32, kind="ExternalOutput")

    with tile.TileContext(nc) as tc:
        with (
            tc.tile_pool(name="consts", bufs=1) as consts,
            tc.tile_pool(name="xpool", bufs=4) as xpool,
            tc.tile_pool(name="xtpool", bufs=8) as xtpool,
            tc.tile_pool(name="gbc", bufs=3) as gbcp,
            tc.tile_pool(name="rows", bufs=2) as rows,
            tc.tile_pool(name="small", bufs=2) as small,
            tc.tile_pool(name="scr", bufs=1) as scr,
            tc.tile_pool(name="tp_ps", bufs=3, space="PSUM") as tp_ps,
            tc.tile_pool(name="sc_ps", bufs=2, space="PSUM") as sc_ps,
            tc.tile_pool(name="sp_ps", bufs=2, space="PSUM") as sp_ps,
            tc.tile_pool(name="ot_ps", bufs=1, space="PSUM") as ot_ps,
        ):
            # ---- constants ----
            ident = consts.tile([P, P], BF16)
            make_identity(nc, ident[:])
            identf = consts.tile([P, P], F32)
            make_identity(nc, identf[:])
            w_f = consts.tile([P, NCHUNK, H], F32)
            nc.sync.dma_start(w_f[:], W_temp.rearrange("(c p) h -> p c h", p=P))
            w_sb = consts.tile([P, NCHUNK, H], BF16)
            nc.vector.tensor_copy(w_sb[:], w_f[:])
            wf_sb = consts.tile([1, H], F32R)
            nc.sync.dma_start(wf_sb[:], W_fea[:])
            bw_sb = consts.tile([H, 1], F32)
            nc.sync.dma_start(bw_sb[:], bw[:, None])
            # uwa = [uw; MC]: memset to MC, DMA uw over rows 0..4
            uwa_f = consts.tile([H + 1, 1], F32)
            nc.vector.memset(uwa_f[:], MC)
            nc.sync.dma_start(uwa_f[:H, 0:1], uw[:, None])
            uwa_sb = consts.tile([H + 1, 1], BF16)
            nc.vector.tensor_copy(uwa_sb[:], uwa_f[:])
            negmc = consts.tile([1, 1], F32)
            nc.vector.memset(negmc[:], -MC)
            # DVE scratch for the stt num ops (dead output)
            stt_scr = scr.tile([P, t_grp], BF16)

            prev = None  # (b, g, xts) pending uw/exp/bcast/num work

            def emit_tail_for(p):
                """uw matmul + exp + bcast + num-stt for a finished group."""
                b_, g_, xts = p
                t0_ = g_ * t_grp
                gi_ = g_
                tanh_b, g_row, g_acc, nacc = batch_rows[b_]
                sp = sp_ps.tile([1, t_grp], F32, tag="sp")
                nc.tensor.matmul(
                    sp[:],
                    uwa_sb[:],
                    tanh_b[:, t0_ : t0_ + t_grp],
                    start=True,
                    stop=True,
                )
                nc.scalar.activation(
                    g_row[:, t0_ : t0_ + t_grp],
                    sp[:],
                    AF.Exp,
                    bias=negmc[:],
                    accum_out=g_acc[:, gi_ : gi_ + 1],
                )
                g_bc = gbcp.tile([P, t_grp], BF16, tag="gbc")
                nc.gpsimd.partition_broadcast(g_bc[:], g_row[:, t0_ : t0_ + t_grp])
                for half in range(4):
                    xt = xts[half]
                    for c2 in range(2):
                        c = half * 2 + c2
                        nc.vector.scalar_tensor_tensor(
                            stt_scr[:],
                            xt[:, c2, :],
                            1.0,
                            g_bc[:],
                            op0=mybir.AluOpType.mult,
                            op1=mybir.AluOpType.mult,
                            accum_out=nacc[:, c, gi_ : gi_ + 1],
                        )

            batch_rows = {}

            for b in range(b_shard):
                # ---- per-batch rows (x DMA of group 0 goes first so the
                # pipeline's head isn't waiting behind the mask DMA) ----
                fea_sb = rows.tile([1, T], F32R, tag="fea")
                tanh_b = rows.tile([H + 1, T], BF16, tag="tanhb")
                g_row = rows.tile([1, T], BF16, tag="grow")
                g_acc = rows.tile([1, n_grp], F32, tag="gacc")
                nacc = rows.tile([P, NCHUNK, n_grp], F32, tag="nacc")
                batch_rows[b] = (tanh_b, g_row, g_acc, nacc)

                for g in range(n_grp):
                    t0 = g * t_grp
                    # casting DMA: f32 HBM -> bf16 SBUF, one group (512 t)
                    x3 = xpool.tile([P, jg, D], BF16, tag="x")
                    nc.gpsimd.dma_start(
                        x3[:],
                        x_temp[b, t0 : t0 + t_grp, :].rearrange(
                            "(j p) d -> p j d", p=P
                        ),
                    )
                    if g == 0:
                        nc.sync.dma_start(fea_sb[:], x_fea[b : b + 1, :])
                        # tanh_b rows 0..4 = tanh(scores); row 5 = mask (0/1
                        # f32), folded via uwa[5]=MC and exp bias=-MC.
                        nc.gpsimd.dma_start(
                            tanh_b[H : H + 1, :], mask[b : b + 1, :]
                        )
                    sc = sc_ps.tile([H, t_grp], F32, tag="sc")
                    nc.tensor.matmul(
                        sc[:],
                        wf_sb[:],
                        fea_sb[:, t0 : t0 + t_grp],
                        start=True,
                        stop=False,
                    )
                    xts = []
                    for half in range(4):
                        tp = tp_ps.tile([P, 2, t_grp], BF16, tag="tp")
                        for c2 in range(2):
                            c = half * 2 + c2
                            for j in range(jg):
                                nc.tensor.transpose(
                                    tp[:, c2, j * P : (j + 1) * P],
                                    x3[:, j, c * P : (c + 1) * P],
                                    ident[:],
                                )
                        xt = xtpool.tile([P, 2, t_grp], BF16, tag="xt")
                        # bf16 pairs viewed as f32 halve the ap length
                        nc.scalar.copy(xt[:].bitcast(F32), tp[:].bitcast(F32))
                        xts.append(xt)
                    if prev is not None:
                        emit_tail_for(prev)
                        prev = None
                    for half in range(4):
                        for c2 in range(2):
                            c = half * 2 + c2
                            nc.tensor.matmul(
                                sc[:],
                                w_sb[:, c, :],
                                xts[half][:, c2, :],
                                start=False,
                                stop=(c == NCHUNK - 1),
                            )
                    nc.scalar.activation(
                        tanh_b[:H, t0 : t0 + t_grp], sc[:], AF.Tanh, bias=bw_sb[:]
                    )
                    prev = (b, g, xts)

                # flush the last group's tail at batch end
                emit_tail_for(prev)
                prev = None

                # ---- batch tail: num8, den, inv, output ----
                num8 = small.tile([P, NCHUNK], F32, tag="num8")
                nc.vector.tensor_reduce(
                    num8[:],
                    nacc[:],
                    axis=mybir.AxisListType.X,
                    op=mybir.AluOpType.add,
                )
                den = small.tile([1, 1], F32, tag="den")
                nc.vector.tensor_reduce(
                    den[:], g_acc[:], axis=mybir.AxisListType.X, op=mybir.AluOpType.add
                )
                inv = small.tile([1, 1], F32, tag="inv")
                nc.vector.reciprocal(inv[:], den[:])
                inv8 = small.tile([NCHUNK, 1], F32, tag="inv8")
                nc.gpsimd.partition_broadcast(inv8[:], inv[:], channels=NCHUNK)
                ot = ot_ps.tile([NCHUNK, P], F32, tag="ot")
                nc.tensor.transpose(ot[:], num8[:], identf[:])
                o_sb = small.tile([NCHUNK, P], F32, tag="osb")
                nc.scalar.activation(
                    o_sb[:], ot[:], AF.Copy, scale=inv8[:]
                )
                nc.sync.dma_start(
                    out[b : b + 1, :].rearrange("o (c p) -> (o c) p", p=P), o_sb[:]
                )

    nc.finalize()
    return nc


_NC_CACHE = {}


def _get_nc(b_shard, T):
    key = (b_shard, T)
    if key not in _NC_CACHE:
        _NC_CACHE[key] = build_kernel(b_shard, T)
    return _NC_CACHE[key]


def kernel(x_temp, x_fea, mask, W_temp, W_fea, bw, uw) -> np.ndarray:
    from concourse.bass_utils import run_bass_kernel_spmd

    B, T, D_ = x_temp.shape
    n_cores = 8
    assert B % n_cores == 0
    bs = B // n_cores

    nc = _get_nc(bs, T)

    x_temp = np.ascontiguousarray(x_temp, dtype=np.float32)
    x_fea = np.ascontiguousarray(x_fea, dtype=np.float32)
    mask_u8 = np.ascontiguousarray(mask).view(np.uint8)
    W_temp = np.ascontiguousarray(W_temp, dtype=np.float32)
    W_fea = np.ascontiguousarray(W_fea, dtype=np.float32)
    bw = np.ascontiguousarray(bw, dtype=np.float32)
    uw = np.ascontiguousarray(uw, dtype=np.float32)

    in_maps = []
    for i in range(n_cores):
        in_maps.append(
            {
                "x_temp": x_temp[i * bs : (i + 1) * bs],
                "x_fea": x_fea[i * bs : (i + 1) * bs],
                "mask": mask_u8[i * bs : (i + 1) * bs],
                "W_temp": W_temp,
                "W_fea": W_fea,
                "bw": bw,
                "uw": uw,
            }
        )

    res = run_bass_kernel_spmd(nc, in_maps, core_ids=list(range(n_cores)))
    return np.concatenate([r["out"] for r in res.results], axis=0)


# revision 26
# speedup vs baseline: 1.2313x; 1.2313x over previous
"""Trainium2 Bass kernel for AttLayer pooling (B=32, T=2048, D=1024, H=5).

Math (equivalent to reference up to exact cancellation of the softmax
normalization): since |tanh| <= 1, scores s[b,t] are bounded by ||uw||_1, so
exp needs no max-subtraction, and the masked renormalization cancels the
softmax denominator:

    out[b,:] = sum_t x[b,t,:] * g[b,t] / sum_t g[b,t]
    g[b,t]   = exp(s[b,t] - 256) with s' = s + 256*mask  (mask fold: masked
               entries get exp(s-256) ~ 1e-110 ~ 0; unmasked are exact)
    s[b,t]   = sum_h tanh( (x @ W)[b,t,h] + fea[b,t]*Wf[h] + bw[h] ) * uw[h]

v2 design (engine budget per core, 4 batches):
  - x arrives as bf16 via GpSimd casting DMA (f32 HBM -> bf16 SBUF), so no
    cast ops on compute engines.  DMA ~94us (streaming floor) is the target
    critical path.
  - PE (~86us): transposes x (bf16, 77ns/128x128), scores GEMM W.T @ xT
    ([5,512] per chunk), fea K=1 matmul, uw matmul with M=1 (lhsT=[6,1])
    giving s' as a psum ROW [1,512], final [128,8] output transpose.
  - ACT (~58us): half the psum->sbuf xT copies, tanh, exp (bias=-256) with
    accum_out giving the per-group denominator for free.
  - DVE (~64us): other half of copies + the whole num reduction:
    scalar_tensor_tensor(xt * g_bcast, accum_out) per (chunk, group) -> num
    columns; num never touches the PE.
  - GpSimd (~30us): casting DMAs, mask u8->f32 DMA straight into tanh row 5,
    per-group partition_broadcast of g.
"""

import sys

sys.path.insert(0, "/opt/trn_rl_repo")

import numpy as np

import concourse.bass as bass
import concourse.mybir as mybir
import concourse.tile as tile
from concourse import bacc
from concourse.masks import make_identity

F32 = mybir.dt.float32
F32R = mybir.dt.float32r
BF16 = mybir.dt.bfloat16
U8 = mybir.dt.uint8
AF = mybir.ActivationFunctionType

P = 128          # partitions / t-tile size
D = 1024         # feature dim
H = 5            # attention hidden dim
NCHUNK = D // P  # 8 d-chunks
MC = 256.0       # mask fold constant: s' = s + MC*mask, exp bias -MC


def build_kernel(b_shard: int, T: int, t_grp: int = 512):
    assert T % t_grp == 0 and t_grp % P == 0
    jg = t_grp // P              # tiles per group (4)
    n_grp = T // t_grp           # groups per batch (4)

    nc = bacc.Bacc(None)

    x_temp = nc.dram_tensor("x_temp", [b_shard, T, D], F32, kind="ExternalInput")
    x_fea = nc.dram_tensor("x_fea", [b_shard, T], F32R, kind="ExternalInput")
    mask = nc.dram_tensor("mask", [b_shard, T], U8, kind="ExternalInput")
    W_temp = nc.dram_tensor("W_temp", [D, H], F32, kind="ExternalInput")
    W_fea = nc.dram_tensor("W_fea", [1, H], F32R, kind="ExternalInput")
    bw = nc.dram_tensor("bw", [H], F32, kind="ExternalInput")
    uw = nc.dram_tensor("uw", [H], F32, kind="ExternalInput")
    out = nc.dram_tensor("out", [b_shard, D], F32, kind="ExternalOutput")

    with tile.TileContext(nc) as tc:
        with (
            tc.tile_pool(name="consts", bufs=1) as consts,
            tc.tile_pool(name="xpool", bufs=4) as xpool,
            tc.tile_pool(name="xtpool", bufs=8) as xtpool,
            tc.tile_pool(name="gbc", bufs=3) as gbcp,
            tc.tile_pool(name="rows", bufs=2) as rows,
            tc.tile_pool(name="small", bufs=2) as small,
            tc.tile_pool(name="scr", bufs=1) as scr,
            tc.tile_pool(name="tp_ps", bufs=3, space="PSUM") as tp_ps,
            tc.tile_pool(name="sc_ps", bufs=2, space="PSUM") as sc_ps,
            tc.tile_pool(name="sp_ps", bufs=2, space="PSUM") as sp_ps,
            tc.tile_pool(name="ot_ps", bufs=1, space="PSUM") as ot_ps,
        ):
            # ---- constants ----
            ident = consts.tile([P, P], BF16)
            make_identity(nc, ident[:])
            identf = consts.tile([P, P], F32)
            make_identity(nc, identf[:])
            w_f = consts.tile([P, NCHUNK, H], F32)
            nc.sync.dma_start(w_f[:], W_temp.rearrange("(c p) h -> p c h", p=P))
            w_sb = consts.tile([P, NCHUNK, H], BF16)
            nc.vector.tensor_copy(w_sb[:], w_f[:])
            wf_sb = consts.tile([1, H], F32R)
            nc.sync.dma_start(wf_sb[:], W_fea[:])
            bw_sb = consts.tile([H, 1], F32)
            nc.sync.dma_start(bw_sb[:], bw[:, None])
            # uwa = [uw; MC]: memset to MC, DMA uw over rows 0..4
            uwa_f = consts.tile([H + 1, 1], F32)
            nc.vector.memset(uwa_f[:], MC)
            nc.sync.dma_start(uwa_f[:H, 0:1], uw[:, None])
            uwa_sb = consts.tile([H + 1, 1], F32R)
            nc.vector.tensor_copy(uwa_sb[:], uwa_f[:])
            negmc = consts.tile([1, 1], F32)
            nc.vector.memset(negmc[:], -MC)
            # DVE scratch for the stt num ops (dead output)
            stt_scr = scr.tile([P, t_grp], BF16)

            prev = None  # (b, g, xts) pending uw/exp/bcast/num work

            def emit_tail_for(p):
                """uw matmul + exp + bcast + num-stt for a finished group."""
                b_, g_, xts = p
                t0_ = g_ * t_grp
                gi_ = g_
                tanh_b, g_row, g_acc, nacc = batch_rows[b_]
                sp = sp_ps.tile([1, t_grp], F32, tag="sp")
                nc.tensor.matmul(
                    sp[:],
                    uwa_sb[:],
                    tanh_b[:, t0_ : t0_ + t_grp],
                    start=True,
                    stop=True,
                )
                nc.scalar.activation(
                    g_row[:, t0_ : t0_ + t_grp],
                    sp[:],
                    AF.Exp,
                    bias=negmc[:],
                    accum_out=g_acc[:, gi_ : gi_ + 1],
                )
                g_bc = gbcp.tile([P, t_grp], BF16, tag="gbc")
                nc.gpsimd.partition_broadcast(g_bc[:], g_row[:, t0_ : t0_ + t_grp])
                for half in range(4):
                    xt = xts[half]
                    for c2 in range(2):
                        c = half * 2 + c2
                        nc.vector.scalar_tensor_tensor(
                            stt_scr[:],
                            xt[:, c2, :],
                            1.0,
                            g_bc[:],
                            op0=mybir.AluOpType.mult,
                            op1=mybir.AluOpType.mult,
                            accum_out=nacc[:, c, gi_ : gi_ + 1],
                        )

            batch_rows = {}

            for b in range(b_shard):
                # ---- per-batch rows (x DMA of group 0 goes first so the
                # pipeline's head isn't waiting behind the mask DMA) ----
                fea_sb = rows.tile([1, T], F32R, tag="fea")
                tanh_b = rows.tile([H + 1, T], F32R, tag="tanhb")
                g_row = rows.tile([1, T], BF16, tag="grow")
                g_acc = rows.tile([1, n_grp], F32, tag="gacc")
                nacc = rows.tile([P, NCHUNK, n_grp], F32, tag="nacc")
                batch_rows[b] = (tanh_b, g_row, g_acc, nacc)

                for g in range(n_grp):
                    t0 = g * t_grp
                    # casting DMA: f32 HBM -> bf16 SBUF, one group (512 t)
                    x3 = xpool.tile([P, jg, D], BF16, tag="x")
                    nc.gpsimd.dma_start(
                        x3[:],
                        x_temp[b, t0 : t0 + t_grp, :].rearrange(
                            "(j p) d -> p j d", p=P
                        ),
                    )
                    if g == 0:
                        nc.sync.dma_start(fea_sb[:], x_fea[b : b + 1, :])
                        # tanh_b rows 0..4 = tanh(scores); row 5 = mask (0/1
                        # f32), folded via uwa[5]=MC and exp bias=-MC.
                        nc.gpsimd.dma_start(
                            tanh_b[H : H + 1, :], mask[b : b + 1, :]
                        )
                    sc = sc_ps.tile([H, t_grp], F32, tag="sc")
                    nc.tensor.matmul(
                        sc[:],
                        wf_sb[:],
                        fea_sb[:, t0 : t0 + t_grp],
                        start=True,
                        stop=False,
                    )
                    xts = []
                    for half in range(4):
                        tp = tp_ps.tile([P, 2, t_grp], BF16, tag="tp")
                        for c2 in range(2):
                            c = half * 2 + c2
                            for j in range(jg):
                                nc.tensor.transpose(
                                    tp[:, c2, j * P : (j + 1) * P],
                                    x3[:, j, c * P : (c + 1) * P],
                                    ident[:],
                                )
                        xt = xtpool.tile([P, 2, t_grp], BF16, tag="xt")
                        # bf16 pairs viewed as f32 halve the ap length
                        nc.scalar.copy(xt[:].bitcast(F32), tp[:].bitcast(F32))
                        xts.append(xt)
                    if prev is not None:
                        emit_tail_for(prev)
                        prev = None
                    for half in range(4):
                        for c2 in range(2):
                            c = half * 2 + c2
                            nc.tensor.matmul(
                                sc[:],
                                w_sb[:, c, :],
                                xts[half][:, c2, :],
                                start=False,
                                stop=(c == NCHUNK - 1),
                            )
                    nc.scalar.activation(
                        tanh_b[:H, t0 : t0 + t_grp], sc[:], AF.Tanh, bias=bw_sb[:]
                    )
                    prev = (b, g, xts)

                # flush the last group's tail at batch end
                emit_tail_for(prev)
                prev = None

                # ---- batch tail: num8, den, inv, output ----
                num8 = small.tile([P, NCHUNK], F32, tag="num8")
                nc.vector.tensor_reduce(
                    num8[:],
                    nacc[:],
                    axis=mybir.AxisListType.X,
                    op=mybir.AluOpType.add,
                )
                den = small.tile([1, 1], F32, tag="den")
                nc.vector.tensor_reduce(
                    den[:], g_acc[:], axis=mybir.AxisListType.X, op=mybir.AluOpType.add
                )
                inv = small.tile([1, 1], F32, tag="inv")
                nc.vector.reciprocal(inv[:], den[:])
                inv8 = small.tile([NCHUNK, 1], F32, tag="inv8")
                nc.gpsimd.partition_broadcast(inv8[:], inv[:], channels=NCHUNK)
                ot = ot_ps.tile([NCHUNK, P], F32, tag="ot")
                nc.tensor.transpose(ot[:], num8[:], identf[:])
                o_sb = small.tile([NCHUNK, P], F32, tag="osb")
                nc.scalar.activation(
                    o_sb[:], ot[:], AF.Copy, scale=inv8[:]
                )
                nc.sync.dma_start(
                    out[b : b + 1, :].rearrange("o (c p) -> (o c) p", p=P), o_sb[:]
                )

    nc.finalize()
    return nc


_NC_CACHE = {}


def _get_nc(b_shard, T):
    key = (b_shard, T)
    if key not in _NC_CACHE:
        _NC_CACHE[key] = build_kernel(b_shard, T)
    return _NC_CACHE[key]


def kernel(x_temp, x_fea, mask, W_temp, W_fea, bw, uw) -> np.ndarray:
    from concourse.bass_utils import run_bass_kernel_spmd

    B, T, D_ = x_temp.shape
    n_cores = 8
    assert B % n_cores == 0
    bs = B // n_cores

    nc = _get_nc(bs, T)

    x_temp = np.ascontiguousarray(x_temp, dtype=np.float32)
    x_fea = np.ascontiguousarray(x_fea, dtype=np.float32)
    mask_u8 = np.ascontiguousarray(mask).view(np.uint8)
    W_temp = np.ascontiguousarray(W_temp, dtype=np.float32)
    W_fea = np.ascontiguousarray(W_fea, dtype=np.float32)
    bw = np.ascontiguousarray(bw, dtype=np.float32)
    uw = np.ascontiguousarray(uw, dtype=np.float32)

    in_maps = []
    for i in range(n_cores):
        in_maps.append(
            {
                "x_temp": x_temp[i * bs : (i + 1) * bs],
                "x_fea": x_fea[i * bs : (i + 1) * bs],
                "mask": mask_u8[i * bs : (i + 1) * bs],
                "W_temp": W_temp,
                "W_fea": W_fea,
                "bw": bw,
                "uw": uw,
            }
        )

    res = run_bass_kernel_spmd(nc, in_maps, core_ids=list(range(n_cores)))
    return np.concatenate([r["out"] for r in res.results], axis=0)


# revision 27
# speedup vs baseline: 1.2395x; 1.0067x over previous
"""Trainium2 Bass kernel for AttLayer pooling (B=32, T=2048, D=1024, H=5).

Math (equivalent to reference up to exact cancellation of the softmax
normalization): since |tanh| <= 1, scores s[b,t] are bounded by ||uw||_1, so
exp needs no max-subtraction, and the masked renormalization cancels the
softmax denominator:

    out[b,:] = sum_t x[b,t,:] * g[b,t] / sum_t g[b,t]
    g[b,t]   = exp(s[b,t] - 256) with s' = s + 256*mask  (mask fold: masked
               entries get exp(s-256) ~ 1e-110 ~ 0; unmasked are exact)
    s[b,t]   = sum_h tanh( (x @ W)[b,t,h] + fea[b,t]*Wf[h] + bw[h] ) * uw[h]

v2 design (engine budget per core, 4 batches):
  - x arrives as bf16 via GpSimd casting DMA (f32 HBM -> bf16 SBUF), so no
    cast ops on compute engines.  DMA ~94us (streaming floor) is the target
    critical path.
  - PE (~86us): transposes x (bf16, 77ns/128x128), scores GEMM W.T @ xT
    ([5,512] per chunk), fea K=1 matmul, uw matmul with M=1 (lhsT=[6,1])
    giving s' as a psum ROW [1,512], final [128,8] output transpose.
  - ACT (~58us): half the psum->sbuf xT copies, tanh, exp (bias=-256) with
    accum_out giving the per-group denominator for free.
  - DVE (~64us): other half of copies + the whole num reduction:
    scalar_tensor_tensor(xt * g_bcast, accum_out) per (chunk, group) -> num
    columns; num never touches the PE.
  - GpSimd (~30us): casting DMAs, mask u8->f32 DMA straight into tanh row 5,
    per-group partition_broadcast of g.
"""

import sys

sys.path.insert(0, "/opt/trn_rl_repo")

import numpy as np

import concourse.bass as bass
import concourse.mybir as mybir
import concourse.tile as tile
from concourse import bacc
from concourse.masks import make_identity

F32 = mybir.dt.float32
F32R = mybir.dt.float32r
BF16 = mybir.dt.bfloat16
U8 = mybir.dt.uint8
AF = mybir.ActivationFunctionType

P = 128          # partitions / t-tile size
D = 1024         # feature dim
H = 5            # attention hidden dim
NCHUNK = D // P  # 8 d-chunks
MC = 256.0       # mask fold constant: s' = s + MC*mask, exp bias -MC


def build_kernel(b_shard: int, T: int, t_grp: int = 512):
    assert T % t_grp == 0 and t_grp % P == 0
    jg = t_grp // P              # tiles per group (4)
    n_grp = T // t_grp           # groups per batch (4)

    nc = bacc.Bacc(None)

    x_temp = nc.dram_tensor("x_temp", [b_shard, T, D], F32, kind="ExternalInput")
    x_fea = nc.dram_tensor("x_fea", [b_shard, T], F32R, kind="ExternalInput")
    mask = nc.dram_tensor("mask", [b_shard, T], U8, kind="ExternalInput")
    W_temp = nc.dram_tensor("W_temp", [D, H], F32, kind="ExternalInput")
    W_fea = nc.dram_tensor("W_fea", [1, H], F32R, kind="ExternalInput")
    bw = nc.dram_tensor("bw", [H], F32, kind="ExternalInput")
    uw = nc.dram_tensor("uw", [H], F32, kind="ExternalInput")
    out = nc.dram_tensor("out", [b_shard, D], F32, kind="ExternalOutput")

    with tile.TileContext(nc) as tc:
        with (
            tc.tile_pool(name="consts", bufs=1) as consts,
            tc.tile_pool(name="xpool", bufs=6) as xpool,
            tc.tile_pool(name="xtpool", bufs=8) as xtpool,
            tc.tile_pool(name="gbc", bufs=4) as gbcp,
            tc.tile_pool(name="rows", bufs=2) as rows,
            tc.tile_pool(name="small", bufs=2) as small,
            tc.tile_pool(name="scr", bufs=1) as scr,
            tc.tile_pool(name="tp_ps", bufs=3, space="PSUM") as tp_ps,
            tc.tile_pool(name="sc_ps", bufs=2, space="PSUM") as sc_ps,
            tc.tile_pool(name="sp_ps", bufs=2, space="PSUM") as sp_ps,
            tc.tile_pool(name="ot_ps", bufs=1, space="PSUM") as ot_ps,
        ):
            # ---- first x chunk: issued before everything else so the
            # Q7/SDMA pipeline starts filling immediately ----
            x3_first = xpool.tile([P, t_grp // P, D], BF16, tag="x")
            nc.gpsimd.dma_start(
                x3_first[:],
                x_temp[0, 0:t_grp, :].rearrange("(j p) d -> p j d", p=P),
            )

            # ---- constants ----
            ident = consts.tile([P, P], BF16)
            make_identity(nc, ident[:])
            identf = consts.tile([P, P], F32)
            make_identity(nc, identf[:])
            w_f = consts.tile([P, NCHUNK, H], F32)
            nc.sync.dma_start(w_f[:], W_temp.rearrange("(c p) h -> p c h", p=P))
            w_sb = consts.tile([P, NCHUNK, H], BF16)
            nc.vector.tensor_copy(w_sb[:], w_f[:])
            wf_sb = consts.tile([1, H], F32R)
            nc.sync.dma_start(wf_sb[:], W_fea[:])
            bw_sb = consts.tile([H, 1], F32)
            nc.sync.dma_start(bw_sb[:], bw[:, None])
            # uwa = [uw; MC]: memset to MC, DMA uw over rows 0..4
            uwa_f = consts.tile([H + 1, 1], F32)
            nc.vector.memset(uwa_f[:], MC)
            nc.sync.dma_start(uwa_f[:H, 0:1], uw[:, None])
            uwa_sb = consts.tile([H + 1, 1], F32R)
            nc.vector.tensor_copy(uwa_sb[:], uwa_f[:])
            negmc = consts.tile([1, 1], F32)
            nc.vector.memset(negmc[:], -MC)
            # DVE scratch for the stt num ops (dead output)
            stt_scr = scr.tile([P, t_grp], BF16)

            prev = None  # (b, g, xts) pending uw/exp/bcast/num work

            def emit_tail_for(p):
                """uw matmul + exp + bcast + num-stt for a finished group."""
                b_, g_, xts = p
                t0_ = g_ * t_grp
                gi_ = g_
                tanh_b, g_row, g_acc, nacc = batch_rows[b_]
                sp = sp_ps.tile([1, t_grp], F32, tag="sp")
                nc.tensor.matmul(
                    sp[:],
                    uwa_sb[:],
                    tanh_b[:, t0_ : t0_ + t_grp],
                    start=True,
                    stop=True,
                )
                nc.scalar.activation(
                    g_row[:, t0_ : t0_ + t_grp],
                    sp[:],
                    AF.Exp,
                    bias=negmc[:],
                    accum_out=g_acc[:, gi_ : gi_ + 1],
                )
                g_bc = gbcp.tile([P, t_grp], BF16, tag="gbc")
                nc.gpsimd.partition_broadcast(g_bc[:], g_row[:, t0_ : t0_ + t_grp])
                for half in range(4):
                    xt = xts[half]
                    for c2 in range(2):
                        c = half * 2 + c2
                        nc.vector.scalar_tensor_tensor(
                            stt_scr[:],
                            xt[:, c2, :],
                            1.0,
                            g_bc[:],
                            op0=mybir.AluOpType.mult,
                            op1=mybir.AluOpType.mult,
                            accum_out=nacc[:, c, gi_ : gi_ + 1],
                        )

            batch_rows = {}

            for b in range(b_shard):
                # ---- per-batch rows (x DMA of group 0 goes first so the
                # pipeline's head isn't waiting behind the mask DMA) ----
                fea_sb = rows.tile([1, T], F32R, tag="fea")
                tanh_b = rows.tile([H + 1, T], F32R, tag="tanhb")
                g_row = rows.tile([1, T], BF16, tag="grow")
                g_acc = rows.tile([1, n_grp], F32, tag="gacc")
                nacc = rows.tile([P, NCHUNK, n_grp], F32, tag="nacc")
                batch_rows[b] = (tanh_b, g_row, g_acc, nacc)

                for g in range(n_grp):
                    t0 = g * t_grp
                    # casting DMA: f32 HBM -> bf16 SBUF, one group (512 t);
                    # group (0,0) was pre-issued before the consts
                    if b == 0 and g == 0:
                        x3 = x3_first
                    else:
                        x3 = xpool.tile([P, jg, D], BF16, tag="x")
                        nc.gpsimd.dma_start(
                            x3[:],
                            x_temp[b, t0 : t0 + t_grp, :].rearrange(
                                "(j p) d -> p j d", p=P
                            ),
                        )
                    if g == 0:
                        nc.sync.dma_start(fea_sb[:], x_fea[b : b + 1, :])
                        # tanh_b rows 0..4 = tanh(scores); row 5 = mask (0/1
                        # f32), folded via uwa[5]=MC and exp bias=-MC.
                        nc.gpsimd.dma_start(
                            tanh_b[H : H + 1, :], mask[b : b + 1, :]
                        )
                    sc = sc_ps.tile([H, t_grp], F32, tag="sc")
                    nc.tensor.matmul(
                        sc[:],
                        wf_sb[:],
                        fea_sb[:, t0 : t0 + t_grp],
                        start=True,
                        stop=False,
                    )
                    xts = []
                    for half in range(4):
                        tp = tp_ps.tile([P, 2, t_grp], BF16, tag="tp")
                        for c2 in range(2):
                            c = half * 2 + c2
                            for j in range(jg):
                                nc.tensor.transpose(
                                    tp[:, c2, j * P : (j + 1) * P],
                                    x3[:, j, c * P : (c + 1) * P],
                                    ident[:],
                                )
                        xt = xtpool.tile([P, 2, t_grp], BF16, tag="xt")
                        # bf16 pairs viewed as f32 halve the ap length
                        nc.scalar.copy(xt[:].bitcast(F32), tp[:].bitcast(F32))
                        xts.append(xt)
                    if prev is not None:
                        emit_tail_for(prev)
                        prev = None
                    for half in range(4):
                        for c2 in range(2):
                            c = half * 2 + c2
                            nc.tensor.matmul(
                                sc[:],
                                w_sb[:, c, :],
                                xts[half][:, c2, :],
                                start=False,
                                stop=(c == NCHUNK - 1),
                            )
                    nc.scalar.activation(
                        tanh_b[:H, t0 : t0 + t_grp], sc[:], AF.Tanh, bias=bw_sb[:]
                    )
                    prev = (b, g, xts)

                # flush the last group's tail at batch end
                emit_tail_for(prev)
                prev = None

                # ---- batch tail: num8, den, inv, output ----
                num8 = small.tile([P, NCHUNK], F32, tag="num8")
                nc.vector.tensor_reduce(
                    num8[:],
                    nacc[:],
                    axis=mybir.AxisListType.X,
                    op=mybir.AluOpType.add,
                )
                den = small.tile([1, 1], F32, tag="den")
                nc.vector.tensor_reduce(
                    den[:], g_acc[:], axis=mybir.AxisListType.X, op=mybir.AluOpType.add
                )
                inv = small.tile([1, 1], F32, tag="inv")
                nc.vector.reciprocal(inv[:], den[:])
                inv8 = small.tile([NCHUNK, 1], F32, tag="inv8")
                nc.gpsimd.partition_broadcast(inv8[:], inv[:], channels=NCHUNK)
                ot = ot_ps.tile([NCHUNK, P], F32, tag="ot")
                nc.tensor.transpose(ot[:], num8[:], identf[:])
                o_sb = small.tile([NCHUNK, P], F32, tag="osb")
                nc.scalar.activation(
                    o_sb[:], ot[:], AF.Copy, scale=inv8[:]
                )
                nc.sync.dma_start(
                    out[b : b + 1, :].rearrange("o (c p) -> (o c) p", p=P), o_sb[:]
                )

    nc.finalize()
    return nc


_NC_CACHE = {}


def _get_nc(b_shard, T):
    key = (b_shard, T)
    if key not in _NC_CACHE:
        _NC_CACHE[key] = build_kernel(b_shard, T)
    return _NC_CACHE[key]


def kernel(x_temp, x_fea, mask, W_temp, W_fea, bw, uw) -> np.ndarray:
    from concourse.bass_utils import run_bass_kernel_spmd

    B, T, D_ = x_temp.shape
    n_cores = 8
    assert B % n_cores == 0
    bs = B // n_cores

    nc = _get_nc(bs, T)

    x_temp = np.ascontiguousarray(x_temp, dtype=np.float32)
    x_fea = np.ascontiguousarray(x_fea, dtype=np.float32)
    mask_u8 = np.ascontiguousarray(mask).view(np.uint8)
    W_temp = np.ascontiguousarray(W_temp, dtype=np.float32)
    W_fea = np.ascontiguousarray(W_fea, dtype=np.float32)
    bw = np.ascontiguousarray(bw, dtype=np.float32)
    uw = np.ascontiguousarray(uw, dtype=np.float32)

    in_maps = []
    for i in range(n_cores):
        in_maps.append(
            {
                "x_temp": x_temp[i * bs : (i + 1) * bs],
                "x_fea": x_fea[i * bs : (i + 1) * bs],
                "mask": mask_u8[i * bs : (i + 1) * bs],
                "W_temp": W_temp,
                "W_fea": W_fea,
                "bw": bw,
                "uw": uw,
            }
        )

    res = run_bass_kernel_spmd(nc, in_maps, core_ids=list(range(n_cores)))
    return np.concatenate([r["out"] for r in res.results], axis=0)
